# revision 20
# baseline (speedup 1.0000x reference)
"""Trainium2 Bass kernel for the LSTM seq2seq autoencoder (layout B).

Strategy:
  - Data-parallel over batch: B=512 -> 64 rows per core on 8 cores.
  - Gates-on-partitions layout: gate preactivations live in one PSUM bank
    [128, 512] = 8 chunks x 64 batch cols, chunk order [i0 i1 f0 f1 o0 o1 g0 g1].
    Each step: 16 (h) + 8 (x, encoder) LDW+MM pairs of N=64 (FWL-eligible
    bf16 weights, LDWEIGHTS hidden behind matmuls) + one rank-8 bias matmul
    (decoder) whose rhs is a block-indicator [8, 512].
  - h^T [128, 2, 64] is produced directly by the DVE h-mul (no per-step
    transposes or PSUM->SBUF copies) and is the rhs of the next step's MMs.
  - Encoder length masking: c frozen by forcing i -> -BIG, f -> +BIG via the
    mbar row of xp; o captured at the freeze step via PE transpose +
    one fused scalar_tensor_tensor (o_acc = o^T_t * e_t + o_acc).
  - Decoder feedback folded: W_comb = Whh + Wih_dec @ out_W.
  - y = out_W @ h + out_b deferred entirely to a batched end-phase GEMM over
    h^T tiles dumped to DRAM each step (DMA engines are otherwise idle).
"""

import numpy as np
import ml_dtypes
from contextlib import ExitStack

import concourse.bass as bass
import concourse.bacc as bacc
import concourse.mybir as mybir
import concourse.tile as tile
from concourse.tile import add_dep_helper
from concourse.bass_utils import run_bass_kernel_spmd

B, T, D, H = 512, 512, 64, 256
G4 = 4 * H  # 1024
NCORES = 8
BL = B // NCORES  # 64
TDEC = T - 1      # 511 decoder steps
BIG = 30000.0
F32 = mybir.dt.float32
BF16 = mybir.dt.bfloat16
BF = ml_dtypes.bfloat16

_PROGRAM = None
LAST_RESULTS = None

# chunk order on the 512 free cols: [i0 i1 f0 f1 o0 o1 g0 g1]
# torch gate rows: i=[0,256) f=[256,512) g=[512,768) o=[768,1024)
CHUNK_ROWS = [(0, 128), (128, 256), (256, 384), (384, 512),
              (768, 896), (896, 1024), (512, 640), (640, 768)]

Sig = mybir.ActivationFunctionType.Sigmoid
Tanh = mybir.ActivationFunctionType.Tanh
Ident = mybir.ActivationFunctionType.Identity
MUL = mybir.AluOpType.mult
ADD = mybir.AluOpType.add


def build_program(t_enc=T, t_dec=TDEC, debug=False):
    nc = bacc.Bacc(None, target_bir_lowering=False)
    f = F32
    if debug:
        gdbg_d = nc.dram_tensor("gdbg", [128, 512], F32, kind="ExternalOutput")
        cdbg_d = nc.dram_tensor("cdbg", [128, 2, BL], F32, kind="ExternalOutput")
        hdbg_d = nc.dram_tensor("hdbg", [128, 2, BL], BF16, kind="ExternalOutput")
        odbg_d = nc.dram_tensor("odbg", [128, 128], BF16, kind="ExternalOutput")
        hbdbg_d = nc.dram_tensor("hbdbg", [128, 2, BL], BF16, kind="ExternalOutput")
    xp_d = nc.dram_tensor("xp", [t_enc, 66, BL], BF16, kind="ExternalInput")
    x0p_d = nc.dram_tensor("x0p", [66, BL], BF16, kind="ExternalInput")
    wxenc_d = nc.dram_tensor("wxenc", [66, 8, 128], BF16, kind="ExternalInput")
    wxdec_d = nc.dram_tensor("wxdec", [66, 8, 128], BF16, kind="ExternalInput")
    whhenc_d = nc.dram_tensor("whhenc", [128, 2, 8, 128], BF16, kind="ExternalInput")
    whhdec_d = nc.dram_tensor("whhdec", [128, 2, 8, 128], BF16, kind="ExternalInput")
    wcomb_d = nc.dram_tensor("wcomb", [128, 2, 8, 128], BF16, kind="ExternalInput")
    bcombT_d = nc.dram_tensor("bcombT", [8, 128], BF16, kind="ExternalInput")
    bencT_d = nc.dram_tensor("bencT", [8, 128], BF16, kind="ExternalInput")
    bdecT_d = nc.dram_tensor("bdecT", [8, 128], BF16, kind="ExternalInput")
    obT_d = nc.dram_tensor("obT", [1, D], BF16, kind="ExternalInput")
    onesy_d = nc.dram_tensor("onesy", [1, 512], BF16, kind="ExternalInput")
    blockones_d = nc.dram_tensor("blockones", [8, 512], BF16, kind="ExternalInput")
    ident_d = nc.dram_tensor("ident", [128, 128], BF16, kind="ExternalInput")
    edup_d = nc.dram_tensor("edup", [128, t_enc], F32, kind="ExternalInput")
    outwT_d = nc.dram_tensor("outwT", [128, 2, D], BF16, kind="ExternalInput")
    outb_d = nc.dram_tensor("outb", [D, 1], F32, kind="ExternalInput")
    hdump_d = nc.dram_tensor("hdump", [128, t_dec, 2, BL], BF16, kind="Internal")
    yt_d = nc.dram_tensor("yt", [t_dec + 1, D, BL], F32, kind="ExternalOutput")

    with ExitStack() as ctx:
        tc = ctx.enter_context(tile.TileContext(nc))
        singles = ctx.enter_context(tc.tile_pool(name="singles", bufs=1))
        xpool = ctx.enter_context(tc.tile_pool(name="xpool", bufs=6))
        work = ctx.enter_context(tc.tile_pool(name="work", bufs=3))
        hpool = ctx.enter_context(tc.tile_pool(name="hpool", bufs=2))
        cpool = ctx.enter_context(tc.tile_pool(name="cpool", bufs=2))
        oap = ctx.enter_context(tc.tile_pool(name="oap", bufs=2))
        ybig = ctx.enter_context(tc.tile_pool(name="ybig", bufs=2))
        gpool = ctx.enter_context(
            tc.tile_pool(name="gpool", bufs=3, space=bass.MemorySpace.PSUM))
        tpp = ctx.enter_context(
            tc.tile_pool(name="tpp", bufs=2, space=bass.MemorySpace.PSUM))
        ypsum = ctx.enter_context(
            tc.tile_pool(name="ypsum", bufs=2, space=bass.MemorySpace.PSUM))

        # ---- persistent constants ----
        s_wxenc = singles.tile([66, 8, 128], BF16)
        nc.sync.dma_start(s_wxenc, wxenc_d[:, :, :])
        s_wxdec = singles.tile([66, 8, 128], BF16)
        nc.sync.dma_start(s_wxdec, wxdec_d[:, :, :])
        s_whhenc = singles.tile([128, 2, 8, 128], BF16)
        nc.sync.dma_start(s_whhenc, whhenc_d[:, :, :, :])
        s_whhdec = singles.tile([128, 2, 8, 128], BF16)
        nc.sync.dma_start(s_whhdec, whhdec_d[:, :, :, :])
        s_wcomb = singles.tile([128, 2, 8, 128], BF16)
        nc.sync.dma_start(s_wcomb, wcomb_d[:, :, :, :])
        s_bcombT = singles.tile([8, 128], BF16)
        nc.sync.dma_start(s_bcombT, bcombT_d[:, :])
        s_bencT = singles.tile([8, 128], BF16)
        nc.sync.dma_start(s_bencT, bencT_d[:, :])
        s_bdecT = singles.tile([8, 128], BF16)
        nc.sync.dma_start(s_bdecT, bdecT_d[:, :])
        s_obT = singles.tile([1, D], BF16)
        nc.sync.dma_start(s_obT, obT_d[:, :])
        s_onesy = singles.tile([1, 512], BF16)
        nc.sync.dma_start(s_onesy, onesy_d[:, :])
        s_bones = singles.tile([8, 512], BF16)
        nc.sync.dma_start(s_bones, blockones_d[:, :])
        s_identb = singles.tile([128, 128], BF16)
        nc.sync.dma_start(s_identb, ident_d[:, :])
        s_edup = singles.tile([128, t_enc], F32)
        nc.sync.dma_start(s_edup, edup_d[:, :])
        s_outwT = singles.tile([128, 2, D], BF16)
        nc.sync.dma_start(s_outwT, outwT_d[:, :, :])
        s_outb = singles.tile([D, 1], f)
        nc.sync.dma_start(s_outb, outb_d[:, :])
        s_x0p = singles.tile([66, BL], BF16)
        nc.sync.dma_start(s_x0p, x0p_d[:, :])

        # ---- initial state ----
        c_prev = singles.tile([128, 2, BL], f, tag="c0")
        nc.vector.memset(c_prev, 0.0)
        hT_prev = singles.tile([128, 2, BL], BF16, tag="h0")
        nc.vector.memset(hT_prev, 0.0)
        o_acc = singles.tile([128, 128], BF16, tag="oacc0")
        nc.vector.memset(o_acc, 0.0)

        def gate_mms(ps, whh, biasT, xlhs=None, xrhs=None):
            """All matmuls of one step into gate PSUM ps [128, 512].

            Exactly one start=True matmul per bank (the rank-8 bias MM, which
            writes the full [128, 512]); everything else accumulates."""
            nc.tensor.matmul(ps, biasT, s_bones,
                             start=True, stop=False, skip_group_check=True)
            if xlhs is not None:
                for m in range(8):
                    nc.tensor.matmul(ps[:, 64 * m:64 * m + 64],
                                     xlhs[:, m, :], xrhs,
                                     start=False, stop=False,
                                     skip_group_check=True)
            # region-major: i/f chunks first (unblocks sig_if), then g, then o
            for ms in ((0, 1, 2, 3), (6, 7), (4, 5)):
                for k in (0, 1):
                    for m in ms:
                        nc.tensor.matmul(ps[:, 64 * m:64 * m + 64],
                                         whh[:, k, m, :], hT_prev[:, k, :],
                                         start=False, stop=(k == 1),
                                         skip_group_check=True)

        def cell(ps, enc_t):
            """LSTM cell elementwise phase. Updates c_prev/hT_prev (+o_acc)."""
            nonlocal c_prev, hT_prev, o_acc
            if_t = work.tile([128, 256], BF16, tag="ift")
            nc.scalar.activation(if_t, ps[:, 0:256], Sig)
            g_t = work.tile([128, 128], BF16, tag="gt")
            nc.scalar.activation(g_t, ps[:, 384:512], Tanh)
            o_t = work.tile([128, 128], BF16, tag="ot")
            nc.scalar.activation(o_t, ps[:, 256:384], Sig)
            c_new = cpool.tile([128, 2, BL], f, tag="c")
            tct = work.tile([128, 2, BL], BF16, tag="tct")
            hT_new = hpool.tile([128, 2, BL], BF16, tag="hT")
            for k in (0, 1):
                sl = slice(64 * k, 64 * k + 64)
                fc = work.tile([128, BL], f, tag=f"fc{k}")
                nc.vector.tensor_mul(fc, if_t[:, 128 + 64 * k:192 + 64 * k],
                                     c_prev[:, k, :])
                ig = work.tile([128, BL], f, tag=f"ig{k}")
                nc.vector.tensor_mul(ig, if_t[:, sl], g_t[:, sl])
                nc.vector.tensor_add(c_new[:, k, :], fc, ig)
                nc.scalar.activation(tct[:, k, :], c_new[:, k, :], Tanh)
                nc.vector.tensor_mul(hT_new[:, k, :], o_t[:, sl], tct[:, k, :])
            if enc_t is not None:
                pending_o[0] = (o_t, enc_t)
            c_prev = c_new
            hT_prev = hT_new

        pending_o = [None]

        def flush_oacc():
            """Deferred o_acc capture: the PE transpose of step t's o is
            emitted after step t+1's matmuls so it never blocks the PE FIFO
            while waiting on sig_o."""
            nonlocal o_acc
            if pending_o[0] is None:
                return
            o_t, t = pending_o[0]
            pending_o[0] = None
            tp = tpp.tile([128, 128], BF16, tag="tp")
            nc.tensor.transpose(tp, o_t, s_identb)
            o_acc2 = oap.tile([128, 128], BF16, tag="oacc")
            nc.vector.scalar_tensor_tensor(
                o_acc2, tp, s_edup[:, t:t + 1], o_acc, MUL, ADD)
            o_acc = o_acc2

        # ================= ENCODER =================
        for t in range(t_enc):
            xp_t = xpool.tile([66, BL], BF16, tag="xp")
            nc.sync.dma_start(xp_t, xp_d[t, :, :])
            ps = gpool.tile([128, 512], f, tag="g")
            gate_mms(ps, s_whhenc, s_bencT, xlhs=s_wxenc, xrhs=xp_t)
            flush_oacc()
            if debug and t == 0:
                gcp = work.tile([128, 512], f, tag="gdbg")
                nc.vector.tensor_copy(gcp, ps)
                nc.sync.dma_start(gdbg_d[:, :], gcp)
            cell(ps, t)

        if debug:
            nc.sync.dma_start(cdbg_d[:, :, :], c_prev)
            nc.sync.dma_start(hdbg_d[:, :, :], hT_prev)
            nc.sync.dma_start(odbg_d[:, :], o_acc)

        flush_oacc()

        # ===== boundary: hT_enc = o_sel^T * tanh(c_final) =====
        tce = work.tile([128, 2, BL], BF16, tag="tct")
        nc.scalar.activation(tce, c_prev, Tanh)
        tpe = tpp.tile([128, 128], BF16, tag="tp")
        nc.tensor.transpose(tpe, o_acc, s_identb)
        o_selT = work.tile([128, 128], BF16, tag="osel")
        nc.vector.tensor_copy(o_selT, tpe)
        hT_b = hpool.tile([128, 2, BL], BF16, tag="hT")
        for k in (0, 1):
            nc.vector.tensor_mul(hT_b[:, k, :], o_selT[:, 64 * k:64 * k + 64],
                                 tce[:, k, :])
        hT_prev = hT_b
        if debug:
            nc.sync.dma_start(hbdbg_d[:, :, :], hT_b)

        # ================= DECODER =================
        for j in range(t_dec):
            ps = gpool.tile([128, 512], f, tag="g")
            if j == 0:
                gate_mms(ps, s_whhdec, s_bdecT, xlhs=s_wxdec, xrhs=s_x0p)
            else:
                gate_mms(ps, s_wcomb, s_bcombT)
            cell(ps, None)
            nc.sync.dma_start(hdump_d[:, j, :, :], hT_prev)

        # ================= Y GEMM PHASE =================
        for s0 in range(0, t_dec, 64):
            n = min(64, t_dec - s0)
            hblk = ybig.tile([128, 64, 2, BL], BF16, tag="hblk")
            nc.sync.dma_start(hblk[:, 0:n, :, :], hdump_d[:, s0:s0 + n, :, :])
            for g0 in range(0, n, 8):
                cnt = min(8, n - g0)
                psy = ypsum.tile([D, 512], f, tag="psy")
                nc.tensor.matmul(psy, s_obT, s_onesy,
                                 start=True, stop=False, skip_group_check=True)
                for k in (0, 1):
                    for tl in range(cnt):
                        nc.tensor.matmul(psy[:, 64 * tl:64 * tl + 64],
                                         s_outwT[:, k, :], hblk[:, g0 + tl, k, :],
                                         start=False, stop=(k == 1),
                                         skip_group_check=True)
                y_sb = work.tile([D, 512], f, tag="ysb")
                nc.scalar.copy(y_sb[:, 0:64 * cnt], psy[:, 0:64 * cnt])
                for tl in range(cnt):
                    nc.sync.dma_start(yt_d[s0 + g0 + tl + 1, :, :],
                                      y_sb[:, 64 * tl:64 * tl + 64])

    nc.compile()
    return nc


def _prep_host(inputs, t_enc=T, t_dec=TDEC):
    """Build per-core in_maps from full inputs (numpy)."""
    x = np.asarray(inputs["input_tensor"], np.float32)
    tgt = np.asarray(inputs["target_tensor"], np.float32)
    lens = np.asarray(inputs["lens"]).astype(np.int64)

    eWih = np.asarray(inputs["enc_Wih"], np.float32)
    eWhh = np.asarray(inputs["enc_Whh"], np.float32)
    eb = (np.asarray(inputs["enc_bih"], np.float32)
          + np.asarray(inputs["enc_bhh"], np.float32))
    dWih = np.asarray(inputs["dec_Wih"], np.float32)
    dWhh = np.asarray(inputs["dec_Whh"], np.float32)
    db = (np.asarray(inputs["dec_bih"], np.float32)
          + np.asarray(inputs["dec_bhh"], np.float32))
    oW = np.asarray(inputs["out_W"], np.float32)
    ob = np.asarray(inputs["out_b"], np.float32)

    wcomb_full = dWhh + dWih @ oW          # [G4, H]
    bcomb = db + dWih @ ob                 # [G4]

    def chunked_x(W, freeze_big):
        # -> [66, 8, 128]: rows 0:64 x-weights^T, row 64 unused, row 65 freeze
        out = np.zeros((66, 8, 128), np.float32)
        for m, (r0, r1) in enumerate(CHUNK_ROWS):
            out[0:64, m, :] = W[r0:r1, :].T
            if freeze_big and m in (0, 1):
                out[65, m, :] = -BIG
            elif freeze_big and m in (2, 3):
                out[65, m, :] = BIG
        return out.astype(BF)

    def chunked_b(b):
        return np.stack([b[r0:r1] for (r0, r1) in CHUNK_ROWS]).astype(BF)

    def chunked_h(W):
        # -> [128, 2, 8, 128]
        out = np.zeros((128, 2, 8, 128), np.float32)
        for m, (r0, r1) in enumerate(CHUNK_ROWS):
            for k in (0, 1):
                out[:, k, m, :] = W[r0:r1, 128 * k:128 * (k + 1)].T
        return out.astype(BF)

    wxenc = chunked_x(eWih, True)
    wxdec = chunked_x(dWih, False)
    whhenc = chunked_h(eWhh)
    whhdec = chunked_h(dWhh)
    wcomb = chunked_h(wcomb_full)
    bcombT = chunked_b(bcomb)
    bencT = chunked_b(eb)
    bdecT = chunked_b(db)
    obT = ob[None, :].astype(BF)
    onesy = np.ones((1, 512), np.float32).astype(BF)
    blockones = np.zeros((8, 512), np.float32)
    for m in range(8):
        blockones[m, 64 * m:64 * m + 64] = 1.0
    blockones = blockones.astype(BF)
    ident = np.eye(128, dtype=np.float32).astype(BF)
    outwT = oW.T.reshape(2, 128, D).transpose(1, 0, 2).astype(BF).copy()
    outb = ob[:, None].astype(np.float32).copy()

    tt = np.arange(t_enc)[None, :]
    in_maps = []
    for c in range(NCORES):
        b0 = c * BL
        xs = x[b0:b0 + BL, :t_enc, :]                # [BL,t,D]
        xp = np.empty((t_enc, 66, BL), np.float32)
        xp[:, 0:D, :] = xs.transpose(1, 2, 0)
        xp[:, D, :] = 1.0
        lc = lens[b0:b0 + BL]
        mbar = (tt >= lc[:, None]).astype(np.float32)   # [BL,t]
        xp[:, D + 1, :] = mbar.T
        efreeze = (tt == (lc[:, None] - 1)).astype(np.float32)  # [BL,t]
        edup = np.concatenate([efreeze, efreeze], 0)    # [128,t]
        x0p = np.zeros((66, BL), np.float32)
        x0p[0:D, :] = tgt[b0:b0 + BL, 0, :].T
        x0p[D, :] = 1.0
        in_maps.append({
            "xp": np.ascontiguousarray(xp).astype(BF),
            "x0p": x0p.astype(BF),
            "wxenc": wxenc, "wxdec": wxdec,
            "whhenc": whhenc, "whhdec": whhdec, "wcomb": wcomb,
            "bcombT": bcombT, "bencT": bencT, "bdecT": bdecT,
            "obT": obT, "onesy": onesy,
            "blockones": blockones, "ident": ident,
            "edup": np.ascontiguousarray(edup),
            "outwT": outwT, "outb": outb,
        })
    return in_maps, lens


def kernel(**inputs) -> np.ndarray:
    global _PROGRAM, LAST_RESULTS
    if _PROGRAM is None:
        _PROGRAM = build_program()
    nc = _PROGRAM
    in_maps, lens = _prep_host(inputs)
    res = run_bass_kernel_spmd(nc, in_maps, core_ids=list(range(NCORES)))
    LAST_RESULTS = res
    out = np.zeros((B, T, D), np.float32)
    for c in range(NCORES):
        yt = res.results[c]["yt"]                      # [T, D, BL]
        out[c * BL:(c + 1) * BL] = yt.transpose(2, 0, 1)
    mask = (np.arange(T)[None, :] < lens[:, None])[:, :, None]
    out *= mask
    out[:, 0, :] = 0.0
    return out


# revision 23
# speedup vs baseline: 1.0366x; 1.0366x over previous
"""Trainium2 Bass kernel for the LSTM seq2seq autoencoder (layout B).

Strategy:
  - Data-parallel over batch: B=512 -> 64 rows per core on 8 cores.
  - Gates-on-partitions layout: gate preactivations live in one PSUM bank
    [128, 512] = 8 chunks x 64 batch cols, chunk order [i0 i1 f0 f1 o0 o1 g0 g1].
    Each step: 16 (h) + 8 (x, encoder) LDW+MM pairs of N=64 (FWL-eligible
    bf16 weights, LDWEIGHTS hidden behind matmuls) + one rank-8 bias matmul
    (decoder) whose rhs is a block-indicator [8, 512].
  - h^T [128, 2, 64] is produced directly by the DVE h-mul (no per-step
    transposes or PSUM->SBUF copies) and is the rhs of the next step's MMs.
  - Encoder length masking: c frozen by forcing i -> -BIG, f -> +BIG via the
    mbar row of xp; o captured at the freeze step via PE transpose +
    one fused scalar_tensor_tensor (o_acc = o^T_t * e_t + o_acc).
  - Decoder feedback folded: W_comb = Whh + Wih_dec @ out_W.
  - y = out_W @ h + out_b deferred entirely to a batched end-phase GEMM over
    h^T tiles dumped to DRAM each step (DMA engines are otherwise idle).
"""

import numpy as np
import ml_dtypes
from contextlib import ExitStack

import concourse.bass as bass
import concourse.bacc as bacc
import concourse.mybir as mybir
import concourse.tile as tile
from concourse.tile import add_dep_helper
from concourse.bass_utils import run_bass_kernel_spmd

B, T, D, H = 512, 512, 64, 256
G4 = 4 * H  # 1024
NCORES = 8
BL = B // NCORES  # 64
TDEC = T - 1      # 511 decoder steps
BIG = 30000.0
F32 = mybir.dt.float32
BF16 = mybir.dt.bfloat16
BF = ml_dtypes.bfloat16

_PROGRAM = None
LAST_RESULTS = None

# chunk order on the 512 free cols: [i0 i1 f0 f1 o0 o1 g0 g1]
# torch gate rows: i=[0,256) f=[256,512) g=[512,768) o=[768,1024)
CHUNK_ROWS = [(0, 128), (128, 256), (256, 384), (384, 512),
              (768, 896), (896, 1024), (512, 640), (640, 768)]

Sig = mybir.ActivationFunctionType.Sigmoid
Tanh = mybir.ActivationFunctionType.Tanh
Ident = mybir.ActivationFunctionType.Identity
MUL = mybir.AluOpType.mult
ADD = mybir.AluOpType.add


def build_program(t_enc=T, t_dec=TDEC, debug=False):
    nc = bacc.Bacc(None, target_bir_lowering=False)
    f = F32
    if debug:
        gdbg_d = nc.dram_tensor("gdbg", [128, 512], F32, kind="ExternalOutput")
        cdbg_d = nc.dram_tensor("cdbg", [128, 2, BL], F32, kind="ExternalOutput")
        hdbg_d = nc.dram_tensor("hdbg", [128, 2, BL], BF16, kind="ExternalOutput")
        odbg_d = nc.dram_tensor("odbg", [128, 128], BF16, kind="ExternalOutput")
        hbdbg_d = nc.dram_tensor("hbdbg", [128, 2, BL], BF16, kind="ExternalOutput")
    xp_d = nc.dram_tensor("xp", [t_enc, 66, BL], BF16, kind="ExternalInput")
    x0p_d = nc.dram_tensor("x0p", [66, BL], BF16, kind="ExternalInput")
    wxenc_d = nc.dram_tensor("wxenc", [66, 8, 128], BF16, kind="ExternalInput")
    wxdec_d = nc.dram_tensor("wxdec", [66, 8, 128], BF16, kind="ExternalInput")
    whhenc_d = nc.dram_tensor("whhenc", [128, 2, 8, 128], BF16, kind="ExternalInput")
    whhdec_d = nc.dram_tensor("whhdec", [128, 2, 8, 128], BF16, kind="ExternalInput")
    wcomb_d = nc.dram_tensor("wcomb", [128, 2, 8, 128], BF16, kind="ExternalInput")
    bcombT_d = nc.dram_tensor("bcombT", [8, 128], BF16, kind="ExternalInput")
    bencT_d = nc.dram_tensor("bencT", [8, 128], BF16, kind="ExternalInput")
    bdecT_d = nc.dram_tensor("bdecT", [8, 128], BF16, kind="ExternalInput")
    obT_d = nc.dram_tensor("obT", [1, D], BF16, kind="ExternalInput")
    onesy_d = nc.dram_tensor("onesy", [1, 512], BF16, kind="ExternalInput")
    blockones_d = nc.dram_tensor("blockones", [8, 512], BF16, kind="ExternalInput")
    ident_d = nc.dram_tensor("ident", [128, 128], BF16, kind="ExternalInput")
    edup_d = nc.dram_tensor("edup", [128, t_enc], F32, kind="ExternalInput")
    outwT_d = nc.dram_tensor("outwT", [128, 2, D], BF16, kind="ExternalInput")
    outb_d = nc.dram_tensor("outb", [D, 1], F32, kind="ExternalInput")
    hdump_d = nc.dram_tensor("hdump", [128, t_dec, 2, BL], BF16, kind="Internal")
    yt_d = nc.dram_tensor("yt", [t_dec + 1, D, BL], F32, kind="ExternalOutput")

    with ExitStack() as ctx:
        tc = ctx.enter_context(tile.TileContext(nc))
        singles = ctx.enter_context(tc.tile_pool(name="singles", bufs=1))
        xpool = ctx.enter_context(tc.tile_pool(name="xpool", bufs=6))
        work = ctx.enter_context(tc.tile_pool(name="work", bufs=3))
        hpool = ctx.enter_context(tc.tile_pool(name="hpool", bufs=2))
        cpool = ctx.enter_context(tc.tile_pool(name="cpool", bufs=2))
        oap = ctx.enter_context(tc.tile_pool(name="oap", bufs=2))
        ybig = ctx.enter_context(tc.tile_pool(name="ybig", bufs=2))
        gpool = ctx.enter_context(
            tc.tile_pool(name="gpool", bufs=3, space=bass.MemorySpace.PSUM))
        tpp = ctx.enter_context(
            tc.tile_pool(name="tpp", bufs=2, space=bass.MemorySpace.PSUM))
        ypsum = ctx.enter_context(
            tc.tile_pool(name="ypsum", bufs=2, space=bass.MemorySpace.PSUM))

        # ---- persistent constants ----
        s_wxenc = singles.tile([66, 8, 128], BF16)
        nc.sync.dma_start(s_wxenc, wxenc_d[:, :, :])
        s_wxdec = singles.tile([66, 8, 128], BF16)
        nc.sync.dma_start(s_wxdec, wxdec_d[:, :, :])
        s_whhenc = singles.tile([128, 2, 8, 128], BF16)
        nc.sync.dma_start(s_whhenc, whhenc_d[:, :, :, :])
        s_whhdec = singles.tile([128, 2, 8, 128], BF16)
        nc.sync.dma_start(s_whhdec, whhdec_d[:, :, :, :])
        s_wcomb = singles.tile([128, 2, 8, 128], BF16)
        nc.sync.dma_start(s_wcomb, wcomb_d[:, :, :, :])
        s_bcombT = singles.tile([8, 128], BF16)
        nc.sync.dma_start(s_bcombT, bcombT_d[:, :])
        s_bencT = singles.tile([8, 128], BF16)
        nc.sync.dma_start(s_bencT, bencT_d[:, :])
        s_bdecT = singles.tile([8, 128], BF16)
        nc.sync.dma_start(s_bdecT, bdecT_d[:, :])
        s_obT = singles.tile([1, D], BF16)
        nc.sync.dma_start(s_obT, obT_d[:, :])
        s_onesy = singles.tile([1, 512], BF16)
        nc.sync.dma_start(s_onesy, onesy_d[:, :])
        s_bones = singles.tile([8, 512], BF16)
        nc.sync.dma_start(s_bones, blockones_d[:, :])
        s_identb = singles.tile([128, 128], BF16)
        nc.sync.dma_start(s_identb, ident_d[:, :])
        s_edup = singles.tile([128, t_enc], F32)
        nc.sync.dma_start(s_edup, edup_d[:, :])
        s_outwT = singles.tile([128, 2, D], BF16)
        nc.sync.dma_start(s_outwT, outwT_d[:, :, :])
        s_outb = singles.tile([D, 1], f)
        nc.sync.dma_start(s_outb, outb_d[:, :])
        s_x0p = singles.tile([66, BL], BF16)
        nc.sync.dma_start(s_x0p, x0p_d[:, :])

        # ---- initial state ----
        c_prev = singles.tile([128, 2, BL], f, tag="c0")
        nc.vector.memset(c_prev, 0.0)
        hT_prev = singles.tile([128, 2, BL], BF16, tag="h0")
        nc.vector.memset(hT_prev, 0.0)
        o_acc = singles.tile([128, 128], BF16, tag="oacc0")
        nc.vector.memset(o_acc, 0.0)

        def chain(insts):
            for a, b in zip(insts[1:], insts[:-1]):
                add_dep_helper(a.ins, b.ins, sync=False, reason="pe-order")

        def gate_mms(ps, whh, biasT, xlhs=None, xrhs=None):
            """All matmuls of one step into gate PSUM ps [128, 512].

            Exactly one start=True matmul per bank (the rank-8 bias MM, which
            writes the full [128, 512]); everything else accumulates. The
            explicit chain pins the scheduler to this PE order: bias + x MMs
            run early (PE-idle window), then h MMs region-major so sig_if's
            i/f chunks complete first."""
            mms = [nc.tensor.matmul(ps, biasT, s_bones,
                                    start=True, stop=False,
                                    skip_group_check=True)]
            if xlhs is not None:
                for m in range(8):
                    mms.append(nc.tensor.matmul(ps[:, 64 * m:64 * m + 64],
                                                xlhs[:, m, :], xrhs,
                                                start=False, stop=False,
                                                skip_group_check=True))
            for ms in ((0, 1, 2, 3), (6, 7), (4, 5)):
                for k in (0, 1):
                    for m in ms:
                        mms.append(nc.tensor.matmul(
                            ps[:, 64 * m:64 * m + 64],
                            whh[:, k, m, :], hT_prev[:, k, :],
                            start=False, stop=(k == 1),
                            skip_group_check=True))
            chain(mms)
            return mms[-1]

        def cell(ps, enc_t):
            """LSTM cell elementwise phase. Updates c_prev/hT_prev (+o_acc)."""
            nonlocal c_prev, hT_prev, o_acc
            if_t = work.tile([128, 256], BF16, tag="ift")
            nc.scalar.activation(if_t, ps[:, 0:256], Sig)
            g_t = work.tile([128, 128], BF16, tag="gt")
            nc.scalar.activation(g_t, ps[:, 384:512], Tanh)
            o_t = work.tile([128, 128], BF16, tag="ot")
            nc.scalar.activation(o_t, ps[:, 256:384], Sig)
            c_new = cpool.tile([128, 2, BL], f, tag="c")
            tct = work.tile([128, 2, BL], BF16, tag="tct")
            hT_new = hpool.tile([128, 2, BL], BF16, tag="hT")
            for k in (0, 1):
                sl = slice(64 * k, 64 * k + 64)
                fc = work.tile([128, BL], f, tag=f"fc{k}")
                nc.vector.tensor_mul(fc, if_t[:, 128 + 64 * k:192 + 64 * k],
                                     c_prev[:, k, :])
                ig = work.tile([128, BL], f, tag=f"ig{k}")
                nc.vector.tensor_mul(ig, if_t[:, sl], g_t[:, sl])
                nc.vector.tensor_add(c_new[:, k, :], fc, ig)
                nc.scalar.activation(tct[:, k, :], c_new[:, k, :], Tanh)
                nc.vector.tensor_mul(hT_new[:, k, :], o_t[:, sl], tct[:, k, :])
            if enc_t is not None:
                pending_o[0] = (o_t, enc_t)
            c_prev = c_new
            hT_prev = hT_new

        pending_o = [None]

        def flush_oacc(after=None):
            """Deferred o_acc capture: the PE transpose of step t's o is
            pinned after step t+1's matmuls so it never blocks the PE FIFO
            while waiting on sig_o."""
            nonlocal o_acc
            if pending_o[0] is None:
                return
            o_t, t = pending_o[0]
            pending_o[0] = None
            tp = tpp.tile([128, 128], BF16, tag="tp")
            tri = nc.tensor.transpose(tp, o_t, s_identb)
            if after is not None:
                add_dep_helper(tri.ins, after.ins, sync=False,
                               reason="defer transpose")
            o_acc2 = oap.tile([128, 128], BF16, tag="oacc")
            nc.vector.scalar_tensor_tensor(
                o_acc2, tp, s_edup[:, t:t + 1], o_acc, MUL, ADD)
            o_acc = o_acc2

        # ================= ENCODER =================
        for t in range(t_enc):
            xp_t = xpool.tile([66, BL], BF16, tag="xp")
            nc.sync.dma_start(xp_t, xp_d[t, :, :])
            ps = gpool.tile([128, 512], f, tag="g")
            last_mm = gate_mms(ps, s_whhenc, s_bencT, xlhs=s_wxenc, xrhs=xp_t)
            flush_oacc(after=last_mm)
            if debug and t == 0:
                gcp = work.tile([128, 512], f, tag="gdbg")
                nc.vector.tensor_copy(gcp, ps)
                nc.sync.dma_start(gdbg_d[:, :], gcp)
            cell(ps, t)

        if debug:
            nc.sync.dma_start(cdbg_d[:, :, :], c_prev)
            nc.sync.dma_start(hdbg_d[:, :, :], hT_prev)
            nc.sync.dma_start(odbg_d[:, :], o_acc)

        flush_oacc()

        # ===== boundary: hT_enc = o_sel^T * tanh(c_final) =====
        tce = work.tile([128, 2, BL], BF16, tag="tct")
        nc.scalar.activation(tce, c_prev, Tanh)
        tpe = tpp.tile([128, 128], BF16, tag="tp")
        nc.tensor.transpose(tpe, o_acc, s_identb)
        o_selT = work.tile([128, 128], BF16, tag="osel")
        nc.vector.tensor_copy(o_selT, tpe)
        hT_b = hpool.tile([128, 2, BL], BF16, tag="hT")
        for k in (0, 1):
            nc.vector.tensor_mul(hT_b[:, k, :], o_selT[:, 64 * k:64 * k + 64],
                                 tce[:, k, :])
        hT_prev = hT_b
        if debug:
            nc.sync.dma_start(hbdbg_d[:, :, :], hT_b)

        # ================= DECODER =================
        for j in range(t_dec):
            ps = gpool.tile([128, 512], f, tag="g")
            if j == 0:
                gate_mms(ps, s_whhdec, s_bdecT, xlhs=s_wxdec, xrhs=s_x0p)
            else:
                gate_mms(ps, s_wcomb, s_bcombT)
            cell(ps, None)
            nc.sync.dma_start(hdump_d[:, j, :, :], hT_prev)

        # ================= Y GEMM PHASE =================
        for s0 in range(0, t_dec, 64):
            n = min(64, t_dec - s0)
            hblk = ybig.tile([128, 64, 2, BL], BF16, tag="hblk")
            nc.sync.dma_start(hblk[:, 0:n, :, :], hdump_d[:, s0:s0 + n, :, :])
            for g0 in range(0, n, 8):
                cnt = min(8, n - g0)
                psy = ypsum.tile([D, 512], f, tag="psy")
                nc.tensor.matmul(psy, s_obT, s_onesy,
                                 start=True, stop=False, skip_group_check=True)
                for k in (0, 1):
                    for tl in range(cnt):
                        nc.tensor.matmul(psy[:, 64 * tl:64 * tl + 64],
                                         s_outwT[:, k, :], hblk[:, g0 + tl, k, :],
                                         start=False, stop=(k == 1),
                                         skip_group_check=True)
                y_sb = work.tile([D, 512], f, tag="ysb")
                nc.scalar.copy(y_sb[:, 0:64 * cnt], psy[:, 0:64 * cnt])
                for tl in range(cnt):
                    nc.sync.dma_start(yt_d[s0 + g0 + tl + 1, :, :],
                                      y_sb[:, 64 * tl:64 * tl + 64])

    nc.compile()
    return nc


def _prep_host(inputs, t_enc=T, t_dec=TDEC):
    """Build per-core in_maps from full inputs (numpy)."""
    x = np.asarray(inputs["input_tensor"], np.float32)
    tgt = np.asarray(inputs["target_tensor"], np.float32)
    lens = np.asarray(inputs["lens"]).astype(np.int64)

    eWih = np.asarray(inputs["enc_Wih"], np.float32)
    eWhh = np.asarray(inputs["enc_Whh"], np.float32)
    eb = (np.asarray(inputs["enc_bih"], np.float32)
          + np.asarray(inputs["enc_bhh"], np.float32))
    dWih = np.asarray(inputs["dec_Wih"], np.float32)
    dWhh = np.asarray(inputs["dec_Whh"], np.float32)
    db = (np.asarray(inputs["dec_bih"], np.float32)
          + np.asarray(inputs["dec_bhh"], np.float32))
    oW = np.asarray(inputs["out_W"], np.float32)
    ob = np.asarray(inputs["out_b"], np.float32)

    wcomb_full = dWhh + dWih @ oW          # [G4, H]
    bcomb = db + dWih @ ob                 # [G4]

    def chunked_x(W, freeze_big):
        # -> [66, 8, 128]: rows 0:64 x-weights^T, row 64 unused, row 65 freeze
        out = np.zeros((66, 8, 128), np.float32)
        for m, (r0, r1) in enumerate(CHUNK_ROWS):
            out[0:64, m, :] = W[r0:r1, :].T
            if freeze_big and m in (0, 1):
                out[65, m, :] = -BIG
            elif freeze_big and m in (2, 3):
                out[65, m, :] = BIG
        return out.astype(BF)

    def chunked_b(b):
        return np.stack([b[r0:r1] for (r0, r1) in CHUNK_ROWS]).astype(BF)

    def chunked_h(W):
        # -> [128, 2, 8, 128]
        out = np.zeros((128, 2, 8, 128), np.float32)
        for m, (r0, r1) in enumerate(CHUNK_ROWS):
            for k in (0, 1):
                out[:, k, m, :] = W[r0:r1, 128 * k:128 * (k + 1)].T
        return out.astype(BF)

    wxenc = chunked_x(eWih, True)
    wxdec = chunked_x(dWih, False)
    whhenc = chunked_h(eWhh)
    whhdec = chunked_h(dWhh)
    wcomb = chunked_h(wcomb_full)
    bcombT = chunked_b(bcomb)
    bencT = chunked_b(eb)
    bdecT = chunked_b(db)
    obT = ob[None, :].astype(BF)
    onesy = np.ones((1, 512), np.float32).astype(BF)
    blockones = np.zeros((8, 512), np.float32)
    for m in range(8):
        blockones[m, 64 * m:64 * m + 64] = 1.0
    blockones = blockones.astype(BF)
    ident = np.eye(128, dtype=np.float32).astype(BF)
    outwT = oW.T.reshape(2, 128, D).transpose(1, 0, 2).astype(BF).copy()
    outb = ob[:, None].astype(np.float32).copy()

    tt = np.arange(t_enc)[None, :]
    in_maps = []
    for c in range(NCORES):
        b0 = c * BL
        xs = x[b0:b0 + BL, :t_enc, :]                # [BL,t,D]
        xp = np.empty((t_enc, 66, BL), np.float32)
        xp[:, 0:D, :] = xs.transpose(1, 2, 0)
        xp[:, D, :] = 1.0
        lc = lens[b0:b0 + BL]
        mbar = (tt >= lc[:, None]).astype(np.float32)   # [BL,t]
        xp[:, D + 1, :] = mbar.T
        efreeze = (tt == (lc[:, None] - 1)).astype(np.float32)  # [BL,t]
        edup = np.concatenate([efreeze, efreeze], 0)    # [128,t]
        x0p = np.zeros((66, BL), np.float32)
        x0p[0:D, :] = tgt[b0:b0 + BL, 0, :].T
        x0p[D, :] = 1.0
        in_maps.append({
            "xp": np.ascontiguousarray(xp).astype(BF),
            "x0p": x0p.astype(BF),
            "wxenc": wxenc, "wxdec": wxdec,
            "whhenc": whhenc, "whhdec": whhdec, "wcomb": wcomb,
            "bcombT": bcombT, "bencT": bencT, "bdecT": bdecT,
            "obT": obT, "onesy": onesy,
            "blockones": blockones, "ident": ident,
            "edup": np.ascontiguousarray(edup),
            "outwT": outwT, "outb": outb,
        })
    return in_maps, lens


def kernel(**inputs) -> np.ndarray:
    global _PROGRAM, LAST_RESULTS
    if _PROGRAM is None:
        _PROGRAM = build_program()
    nc = _PROGRAM
    in_maps, lens = _prep_host(inputs)
    res = run_bass_kernel_spmd(nc, in_maps, core_ids=list(range(NCORES)))
    LAST_RESULTS = res
    out = np.zeros((B, T, D), np.float32)
    for c in range(NCORES):
        yt = res.results[c]["yt"]                      # [T, D, BL]
        out[c * BL:(c + 1) * BL] = yt.transpose(2, 0, 1)
    mask = (np.arange(T)[None, :] < lens[:, None])[:, :, None]
    out *= mask
    out[:, 0, :] = 0.0
    return out
